# revision 1
# baseline (speedup 1.0000x reference)
"""Trainium2 Bass kernel for nn_ContextualCritic (4-layer strided conv + segment mean).

Self-contained: kernel(**inputs) -> np.ndarray [2B, 8192].

Design v3 (per core, data-parallel over 8 cores, 512 images each):
 - All matmul operands bf16 (1 cyc/col on the PE; fp32 streams at ~2),
   fp32 PSUM accumulation, evictions write bf16 back to SBUF.
 - Single fused pipeline: L1 -> L2 -> L3 stay SBUF-resident; L4 runs per
   32-image group. All weights preloaded to SBUF (no per-group weight DMA).
 - Parity-split activation layouts: each conv reads its input with stride-2
   taps, so inputs are stored as 4 parity planes; every matmul moving-operand
   AP then has a contiguous, 16B-aligned innermost run (PE SBUF fetch is
   16B-chunked -- strided runs halve streaming rate).
 - L1 im2col column order is parity-grouped on the host so the L1 eviction
   into the parity-split l2i is a single linear AP.
 - L3/L4 PSUM column order is image-minor, making the l4i parity planes
   image-minor (64B contiguous runs) at no eviction cost.
 - L2 (64->128): 25 taps as 12 interleaved K=64 matmul pairs on PE row groups
   (0,0)/(64,0) into two PSUM banks + odd tap as K=128 with zeroed high rows;
   DVE adds the banks, ACT applies LeakyReLU into the parity-split l3i.
 - L1 eviction is split between ACT and DVE (2-op lrelu) to keep the l1ps
   PSUM WAR chain off the tensor engine's critical path.
 - Segment mean on host from [N,8192] features (sorted segment ids).
"""
import numpy as np

BLK = 8        # images per block
GRP = 32       # images per L4 group (nimg = ngrp*GRP)
NCORES = 8

_CACHE = {}


def _build_program(nimg, zero_bias):
    from concourse import bacc, mybir
    import concourse.tile as tile

    BF16 = mybir.dt.bfloat16
    F32 = mybir.dt.float32
    LRELU = mybir.ActivationFunctionType.Prelu
    ADD = mybir.AluOpType.add
    MAX = mybir.AluOpType.max

    nblk = nimg // BLK

    nc = bacc.Bacc(None, target_bir_lowering=False)

    icd = nc.dram_tensor("ic", [75, nimg * 1024], BF16, kind="ExternalInput")
    w1d = nc.dram_tensor("w1", [128, 128], BF16, kind="ExternalInput")
    w2d = nc.dram_tensor("w2", [128, 25 * 128], BF16, kind="ExternalInput")
    w3d = nc.dram_tensor("w3", [128, 2 * 25 * 128], BF16, kind="ExternalInput")
    w4d = nc.dram_tensor("w4", [128, 2 * 25 * 512], BF16, kind="ExternalInput")
    b1d = nc.dram_tensor("b1", [128, 1], F32, kind="ExternalInput")
    b2d = nc.dram_tensor("b2", [128, 1], F32, kind="ExternalInput")
    b3d = nc.dram_tensor("b3", [128, 2], F32, kind="ExternalInput")
    b4d = nc.dram_tensor("b4", [128, 4], F32, kind="ExternalInput")
    fd = nc.dram_tensor("f", [128, 4, nimg, 16], F32, kind="ExternalOutput")

    with tile.TileContext(nc) as tc:
        with tc.tile_pool(name="const", bufs=1) as cst, \
             tc.tile_pool(name="work", bufs=1) as wk, \
             tc.tile_pool(name="ps", bufs=2, space="PSUM") as ps2, \
             tc.tile_pool(name="ps1", bufs=1, space="PSUM") as ps1, \
             tc.tile_pool(name="stg", bufs=2) as stg:
            w1t = cst.tile([128, 128], BF16)
            nc.sync.dma_start(w1t[:], w1d[:, :])
            w2t = cst.tile([128, 25 * 128], BF16)
            nc.sync.dma_start(w2t[:], w2d[:, :])
            w3t = cst.tile([128, 2 * 25 * 128], BF16)
            nc.sync.dma_start(w3t[:], w3d[:, :])
            w4t = cst.tile([128, 2 * 25 * 512], BF16)
            nc.sync.dma_start(w4t[:], w4d[:, :])
            b1t = cst.tile([128, 1], F32)
            nc.sync.dma_start(b1t[:], b1d[:, :])
            b2t = cst.tile([128, 1], F32)
            nc.sync.dma_start(b2t[:], b2d[:, :])
            b3t = cst.tile([128, 2], F32)
            nc.sync.dma_start(b3t[:], b3d[:, :])
            b4t = cst.tile([128, 4], F32)
            nc.sync.dma_start(b4t[:], b4d[:, :])
            a2t = cst.tile([128, 1], F32)
            nc.vector.memset(a2t[:], 0.2)

            # parity-split working tiles (pr, pc outermost)
            icT = [wk.tile([128, BLK * 1024], BF16, name=f"ic{i}")
                   for i in range(2)]
            l2iT = [wk.tile([128, 2, 2, BLK, 18, 18], BF16, name=f"l2i{i}")
                    for i in range(2)]
            l3iT = [wk.tile([128, 2, 2, 10, 10, BLK], BF16, name=f"l3i{i}")
                    for i in range(2)]
            l4iT = [wk.tile([128, 2, 2, 6, 6, GRP], BF16, name=f"l4i{i}")
                    for i in range(2)]
            for i in range(2):
                nc.vector.memset(icT[i][64:128, :], 0.0)
                nc.vector.memset(l2iT[i][:], 0.0)
                nc.vector.memset(l3iT[i][:], 0.0)
                nc.vector.memset(l4iT[i][:], 0.0)

            for blk in range(nblk):
                ic = icT[blk % 2]
                l2i = l2iT[blk % 2]
                l3i = l3iT[blk % 2]
                c0 = blk * BLK * 1024
                nc.sync.dma_start(ic[0:38, :], icd[0:38, c0:c0 + BLK * 1024])
                nc.sync.dma_start(ic[38:75, :], icd[38:75, c0:c0 + BLK * 1024])
                # ---- L1: 16 psum blocks of 512 out px (half image each),
                # host ic column order is [pr, pc, r2(8), c2(16)] per half.
                for psb in range(16):
                    img, h = psb // 2, psb % 2
                    ps = ps2.tile([128, 2, 2, 8, 16], F32, tag="l1ps")
                    nc.tensor.matmul(ps[:], w1t[:, :],
                                     ic[:, psb * 512:(psb + 1) * 512],
                                     start=True, stop=True)
                    dst = l2i[:, :, :, img, 1 + 8 * h:9 + 8 * h, 1:17]
                    if zero_bias and psb % 8 >= 5:
                        # DVE 2-op LeakyReLU eviction (bias known zero)
                        tmp = stg.tile([128, 512], F32, tag="l1tmp")
                        nc.vector.tensor_scalar_mul(
                            tmp[:].rearrange("p (a b r c) -> p a b r c",
                                             a=2, b=2, r=8),
                            ps[:], 0.2)
                        nc.vector.tensor_tensor(
                            dst, ps[:],
                            tmp[:].rearrange("p (a b r c) -> p a b r c",
                                             a=2, b=2, r=8), op=MAX)
                    else:
                        nc.scalar.activation(dst, ps[:], LRELU,
                                             bias=b1t[:, :], alpha=a2t[:, :])
                # ---- L2: 4 psum blocks (2 images each), raster psum order
                for psb in range(4):
                    j0 = 2 * psb
                    psA = ps1.tile([128, 2, 16, 16], F32, tag="l2psA")
                    psB = ps1.tile([128, 2, 16, 16], F32, tag="l2psB")
                    for i in range(12):
                        tA, tB = 2 * i, 2 * i + 1
                        ka, wa = tA // 5, tA % 5
                        kb, wb = tB // 5, tB % 5
                        nc.tensor.matmul(
                            psA[:], w2t[0:64, tA * 128:(tA + 1) * 128],
                            l2i[0:64, ka % 2, wa % 2, j0:j0 + 2,
                                ka // 2:ka // 2 + 16, wa // 2:wa // 2 + 16],
                            start=(i == 0), stop=False)
                        nc.tensor.matmul(
                            psB[:], w2t[64:128, tB * 128:(tB + 1) * 128],
                            l2i[64:128, kb % 2, wb % 2, j0:j0 + 2,
                                kb // 2:kb // 2 + 16, wb // 2:wb // 2 + 16],
                            start=(i == 0), stop=(i == 11),
                            tile_position=(64, 0))
                    # tap 24 = (4,4) as K=128 (high weight rows zero on host)
                    nc.tensor.matmul(
                        psA[:], w2t[:, 24 * 128:25 * 128],
                        l2i[:, 0, 0, j0:j0 + 2, 2:18, 2:18],
                        start=False, stop=True)
                    tb = stg.tile([128, 2, 16, 16], F32, tag="l2tb")
                    nc.vector.tensor_copy(tb[:], psB[:])
                    st = stg.tile([128, 2, 16, 16], F32, tag="l2st")
                    nc.vector.tensor_tensor(st[:], psA[:], tb[:], op=ADD)
                    # evict into parity-split l3i (img-minor): 4 ACTs
                    for pr in range(2):
                        for pc in range(2):
                            nc.scalar.activation(
                                l3i[:, pr, pc, 1:9, 1:9, j0:j0 + 2]
                                .rearrange("p r c i -> p i r c"),
                                st[:, :, pr::2, pc::2], LRELU,
                                bias=b2t[:, :], alpha=a2t[:, :])
                # ---- L3: 2 ci planes x 25 taps, psum order [r, c, img]
                sb4 = blk % 4
                for cp in range(2):
                    ps3 = ps2.tile([128, 8, 8, BLK], F32, tag="l3ps")
                    for tap in range(25):
                        kh, kw = tap // 5, tap % 5
                        nc.tensor.matmul(
                            ps3[:],
                            w3t[:, (cp * 25 + tap) * 128:
                                (cp * 25 + tap + 1) * 128],
                            l3i[:, kh % 2, kw % 2, kh // 2:kh // 2 + 8,
                                kw // 2:kw // 2 + 8, :],
                            start=(tap == 0), stop=(tap == 24))
                    for pr in range(2):
                        for pc in range(2):
                            nc.scalar.activation(
                                l4iT[cp][:, pr, pc, 1:5, 1:5,
                                         sb4 * BLK:(sb4 + 1) * BLK],
                                ps3[:, pr::2, pc::2, :], LRELU,
                                bias=b3t[:, cp:cp + 1], alpha=a2t[:, :])
                # ---- L4 over the completed 32-image group, q-pair passes
                if sb4 == 3:
                    grp = blk // 4
                    for half in range(2):
                        p4 = [ps1.tile([128, 4, 4, GRP], F32, name=f"p4_{qi}",
                                       tag=f"l4ps{qi}") for qi in range(2)]
                        for i4 in range(50):
                            cip, tap = i4 // 25, i4 % 25
                            kh, kw = tap // 5, tap % 5
                            for qi in range(2):
                                q = 2 * half + qi
                                w0 = (cip * 25 + tap) * 512 + q * 128
                                nc.tensor.matmul(
                                    p4[qi][:],
                                    w4t[:, w0:w0 + 128],
                                    l4iT[cip][:, kh % 2, kw % 2,
                                              kh // 2:kh // 2 + 4,
                                              kw // 2:kw // 2 + 4, :],
                                    start=(i4 == 0), stop=(i4 == 49))
                        for qi in range(2):
                            q = 2 * half + qi
                            fo = stg.tile([128, GRP, 16], F32, tag="fo")
                            nc.scalar.activation(
                                fo[:], p4[qi][:].rearrange(
                                    "p r c i -> p i (r c)"),
                                LRELU, bias=b4t[:, q:q + 1], alpha=a2t[:, :])
                            nc.sync.dma_start(
                                fd[:, q, grp * GRP:(grp + 1) * GRP, :],
                                fo[:])
    nc.compile()
    return nc


def _prep_inputs(x, W1, b1, W2, b2, W3, b3, W4, b4, nimg):
    """Host preprocessing -> per-core in_maps (shared weight arrays)."""
    import ml_dtypes
    bf16 = ml_dtypes.bfloat16
    f32 = np.float32
    n = x.shape[0]
    ncores = n // nimg
    xpad = np.pad(np.asarray(x, dtype=f32), ((0, 0), (0, 0), (2, 2), (2, 2)))
    s = xpad.strides
    v = np.lib.stride_tricks.as_strided(
        xpad, shape=(n, 3, 5, 5, 32, 32),
        strides=(s[0], s[1], s[2], s[3], 2 * s[2], 2 * s[3]))
    # column order per image: [h(2), pr(2), pc(2), r2(8), c2(16)]
    # row 32 = h*16 + r2*2 + pr ; col 32 = c2*2 + pc
    vr = v.reshape(n, 3, 5, 5, 2, 8, 2, 16, 2)      # rows->(h,r2,pr) cols->(c2,pc)
    vp = vr.transpose(1, 2, 3, 0, 4, 6, 8, 5, 7)    # [3,5,5,n,h,pr,pc,r2,c2]
    ic_all = np.ascontiguousarray(
        vp.reshape(75, n, 1024).astype(bf16))

    w1l = np.ascontiguousarray(
        np.asarray(W1, f32).transpose(1, 2, 3, 0).reshape(75, 64))
    w1h = np.zeros((128, 128), f32)
    w1h[0:75, 0:64] = w1l
    w1h[0:75, 64:128] = w1l
    b1h = np.concatenate([b1, b1]).astype(f32).reshape(128, 1)

    w2h = np.zeros((128, 25 * 128), f32)
    for t in range(25):
        kh, kw = t // 5, t % 5
        lhs = np.asarray(W2, f32)[:, :, kh, kw].T                # [64,128]
        w2h[0:64, t * 128:(t + 1) * 128] = lhs
        if t < 24:
            w2h[64:128, t * 128:(t + 1) * 128] = lhs
    b2h = np.asarray(b2, f32).reshape(128, 1)

    w3h = np.zeros((128, 2 * 25 * 128), f32)
    for cp in range(2):
        for t in range(25):
            kh, kw = t // 5, t % 5
            w3h[:, (cp * 25 + t) * 128:(cp * 25 + t + 1) * 128] = \
                np.asarray(W3, f32)[cp * 128:(cp + 1) * 128, :, kh, kw].T
    b3h = np.ascontiguousarray(
        np.asarray(b3, f32).reshape(2, 128).T)                   # [128,2]

    # w4 SBUF-resident layout: [(cip*25+tap)*512 + q*128 + m] columns
    w4h = np.zeros((128, 2 * 25 * 512), f32)
    for cip in range(2):
        for t in range(25):
            kh, kw = t // 5, t % 5
            w4h[:, (cip * 25 + t) * 512:(cip * 25 + t + 1) * 512] = \
                np.asarray(W4, f32)[:, cip * 128:(cip + 1) * 128, kh, kw].T
    b4h = np.ascontiguousarray(
        np.asarray(b4, f32).reshape(4, 128).T)                   # [128,4]

    w1h = w1h.astype(bf16)
    w2h = w2h.astype(bf16)
    w3h = w3h.astype(bf16)
    w4h = w4h.astype(bf16)

    in_maps = []
    for c in range(ncores):
        ic = np.ascontiguousarray(
            ic_all[:, c * nimg:(c + 1) * nimg, :].reshape(75, nimg * 1024))
        in_maps.append({"ic": ic, "w1": w1h, "w2": w2h, "w3": w3h,
                        "w4": w4h, "b1": b1h, "b2": b2h, "b3": b3h,
                        "b4": b4h})
    return in_maps


def _run(inputs, trace=False, nimg=512, ncores=NCORES):
    from concourse.bass_utils import run_bass_kernel_spmd

    zero_bias = not np.any(np.asarray(inputs["b1"]))
    key = (nimg, ncores, zero_bias)
    if key not in _CACHE:
        _CACHE[key] = _build_program(nimg, zero_bias)
    nc = _CACHE[key]

    in_maps = _prep_inputs(
        inputs["x"], inputs["W1"], inputs["b1"], inputs["W2"], inputs["b2"],
        inputs["W3"], inputs["b3"], inputs["W4"], inputs["b4"], nimg)

    res = run_bass_kernel_spmd(nc, in_maps, core_ids=list(range(ncores)),
                               trace=trace)
    feats = np.concatenate(
        [r["f"].transpose(2, 1, 0, 3).reshape(nimg, 8192)
         for r in res.results], axis=0)                          # [N, 8192]
    return feats, res


def kernel(**inputs):
    x = np.asarray(inputs["x"])
    n = x.shape[0]
    nimg = n // NCORES
    feats, _ = _run(inputs, trace=False, nimg=nimg)

    if int(np.asarray(inputs.get("is_local", 1))) == 0:
        return feats.astype(np.float32)

    batch_size = int(np.asarray(inputs["batch_size"]))
    seg = np.asarray(inputs["f_obj_to_img"]).astype(np.int64)
    nh = n // 2
    fake, real = feats[:nh], feats[nh:]
    counts = np.bincount(seg, minlength=batch_size).astype(np.float32)
    denom = np.maximum(counts, 1.0)[:, None]
    fsum = np.zeros((batch_size, 8192), np.float32)
    rsum = np.zeros((batch_size, 8192), np.float32)
    np.add.at(fsum, seg, fake)
    np.add.at(rsum, seg, real)
    favg = np.where((counts > 0)[:, None], fsum / denom, 0.0)
    ravg = np.where((counts > 0)[:, None], rsum / denom, 0.0)
    return np.concatenate([favg, ravg], axis=0).astype(np.float32)

